# revision 1
# baseline (speedup 1.0000x reference)
"""BiDAF attention kernel for Trainium2, data-parallel over batch on 8 NeuronCores.

Reference math (per batch b):
    S = (ctx * w_m) @ query^T + ctx@w_c [:,None] + query@w_q [None,:]   [C, Q]
    a = softmax(S, axis=q);       attended_query    aq = a @ query       [C, H]
    m = max(S, axis=q); bweights = softmax(m, axis=c)
    attended_context ac = bweights @ ctx                                  [H]
    G = concat([ctx, aq, ctx*aq, ctx*ac[None,:]], axis=-1)               [C, 4H]

Per-core layout (4 batches each):
  - ctx loaded natural [c, d] as [128, 4, 512]; ctx^T built with 16 PE
    transposes/batch (fp32 has no DMA transpose); evacuations alternate
    ScalarE/VectorE.
  - S accumulated in PSUM [128c, 65] per c-tile (two half-batch banks so
    stats overlap the remaining S matmuls): 4 matmuls with lhsT=ctxT
    chunk, rhs=[qT*w_m | w_c], plus one K=1 matmul with lhsT=ones,
    rhs=[s_q | 0] -> column 64 holds s_c, columns 0:63 hold s_m + s_q.
    exp runs on ScalarE with bias=s_c and accum_out giving the softmax
    denominators for free.  Softmax skips max-subtraction (|S| <= ~8
    for this input distribution, exp is safe in fp32).
  - a is renormalized lazily: the aq matmul uses unnormalized exp(S),
    and the PSUM evacuation multiplies by 1/rowsum.
  - b-path: em_t = exp(max_q S_t + s_c_t) per tile feeds the ac matmul
    accumulation immediately.  The ac matmul uses the em column with a
    stride-0 free dim as lhsT, so every output partition computes the
    same sum_c em*ctx row -- attended_context lands already broadcast
    to [128, H] with no separate broadcast step.  The partition sum of
    em runs on the otherwise idle GpSimd.
  - Stores: G[:, 0:H] streams straight from the ctx input tile (fills
    the DMA ramp), G[:, H:3H] per c-tile on the SP HWDGE ring as soon
    as G3 lands, G[:, 3H:4H] per c-tile via GpSimd SWDGE once the
    batch-global attended_context is ready.
  - Emission is software-pipelined with a 1-batch skew (early half of
    batch b+1 ahead of the late half of batch b) so no engine stream
    head-blocks the next batch's independent work.
"""

import numpy as np
from contextlib import ExitStack

import concourse.bass as bass
import concourse.bacc as bacc
import concourse.bass_isa as bass_isa
import concourse.tile as tile
from concourse import mybir
from concourse.bass_utils import run_bass_kernel_spmd
from concourse.masks import make_identity

F32 = mybir.dt.float32
F32R = mybir.dt.float32r
AF = mybir.ActivationFunctionType

B, C, Q, H = 32, 512, 64, 512
NCORES = 8
BPC = B // NCORES  # batches per core
CT = C // 128  # c tiles
KT = H // 128  # contraction chunks


def build_nc():
    nc = bacc.Bacc("TRN2", target_bir_lowering=False, debug=False)
    ctx_d = nc.dram_tensor("context", [BPC, C, H], F32, kind="ExternalInput")
    qry_d = nc.dram_tensor("query", [BPC, Q, H], F32, kind="ExternalInput")
    w_d = nc.dram_tensor("W", [3 * H], F32, kind="ExternalInput")
    g_d = nc.dram_tensor("G", [BPC, C, 4 * H], F32, kind="ExternalOutput")

    with tile.TileContext(nc) as tc, ExitStack() as ex:
        consts = ex.enter_context(tc.tile_pool(name="consts", bufs=1))
        ctx_pool = ex.enter_context(tc.tile_pool(name="ctx", bufs=4))
        ctxT_pool = ex.enter_context(tc.tile_pool(name="ctxT", bufs=2))
        q_pool = ex.enter_context(tc.tile_pool(name="q", bufs=3))
        small_pool = ex.enter_context(tc.tile_pool(name="small", bufs=3))
        g_pool = ex.enter_context(tc.tile_pool(name="g", bufs=8))
        ps_ctxT = ex.enter_context(tc.tile_pool(name="ps_ctxT", bufs=2, space="PSUM"))
        ps_S = ex.enter_context(tc.tile_pool(name="ps_S", bufs=2, space="PSUM"))
        ps_aq = ex.enter_context(tc.tile_pool(name="ps_aq", bufs=1, space="PSUM"))
        ps_small = ex.enter_context(tc.tile_pool(name="ps_small", bufs=2, space="PSUM"))
        ps_b = ex.enter_context(tc.tile_pool(name="ps_b", bufs=1, space="PSUM"))

        # --- constants ---
        wsb = consts.tile([128, 12], F32)  # cols 0:4 w_c, 4:8 w_q, 8:12 w_m chunks
        ident = consts.tile([128, 128], F32)
        make_identity(nc, ident)
        ones_row = consts.tile([1, 128], F32)
        nc.vector.memset(ones_row, 1.0)

        def stage_early(b):
            st = {}
            ctx_v = ctx_d[b].rearrange("(t p) d -> p t d", p=128)  # [128, CT, H]
            st["g_v"] = g_v = g_d[b].rearrange("(t p) f -> p t f", p=128)

            # --- loads; G1 = ctx streams straight back out ---
            ctx_sb = ctx_pool.tile([128, CT, H], F32, tag="ctx_sb", name=f"ctx_sb{b}")
            st["ctx_sb"] = ctx_sb
            nc.sync.dma_start(out=ctx_sb, in_=ctx_v)
            q_sb = q_pool.tile([Q, H], F32, tag="q_sb")
            nc.sync.dma_start(out=q_sb, in_=qry_d[b])
            nc.sync.dma_start(out=g_v[:, :, 0:H], in_=ctx_sb)
            if b == 0:
                nc.sync.dma_start(out=wsb, in_=w_d[:].rearrange("(g p) -> p g", p=128))
            st["q_sb"] = q_sb

            # --- query transpose + scaled rhs build ---
            qt_ps = ps_small.tile([128, KT * Q], F32, tag="ps_misc")
            for k in range(KT):
                nc.tensor.transpose(
                    qt_ps[:, k * Q : (k + 1) * Q],
                    q_sb[:, k * 128 : (k + 1) * 128],
                    ident[:Q, :Q],
                )
            qT_sb = small_pool.tile([128, KT * Q], F32, tag="qT_sb")
            nc.vector.tensor_copy(qT_sb, qt_ps)

            # rhs_ext[:, k, 0:64] = qT_k * w_m_k ; [:, k, 64] = w_c_k
            rhs_ext = small_pool.tile([128, KT, Q + 1], F32, tag="rhs_ext")
            for k in range(KT):
                nc.vector.tensor_scalar_mul(
                    out=rhs_ext[:, k, 0:Q],
                    in0=qT_sb[:, k * Q : (k + 1) * Q],
                    scalar1=wsb[:, 8 + k : 9 + k],
                )
                nc.vector.tensor_copy(rhs_ext[:, k, Q : Q + 1], wsb[:, k : k + 1])

            # s_q^T = w_q . qT  -> [1, Q]
            sq_ps = ps_small.tile([1, Q], F32, tag="ps_misc")
            for k in range(KT):
                nc.tensor.matmul(
                    sq_ps,
                    lhsT=wsb[:, 4 + k : 5 + k],
                    rhs=qT_sb[:, k * Q : (k + 1) * Q],
                    start=(k == 0),
                    stop=(k == KT - 1),
                )
            rhs_sq = small_pool.tile([1, Q + 1], F32, tag="rhs_sq")
            nc.vector.memset(rhs_sq, 0.0)
            nc.vector.tensor_copy(rhs_sq[:, 0:Q], sq_ps)

            # --- context transpose: ctxT_sb[:, k, :] = ctx[:, :, k-chunk]^T ---
            ctxT_sb = ctxT_pool.tile([128, KT, C], F32, tag="ctxT_sb")
            for k in range(KT):
                tps = ps_ctxT.tile([128, C], F32, tag="ps_ctxT")
                for t in range(CT):
                    nc.tensor.transpose(
                        tps[:, t * 128 : (t + 1) * 128],
                        ctx_sb[:, t, k * 128 : (k + 1) * 128],
                        ident,
                    )
                if k % 2 == 0:
                    nc.scalar.copy(out=ctxT_sb[:, k, :], in_=tps)
                else:
                    nc.vector.tensor_copy(ctxT_sb[:, k, :], tps)

            # --- S matmuls: S[c, 0:64] = s_m + s_q ; S[c, 64] = s_c ---
            s_lo = ps_S.tile([128, 2, Q + 1], F32, tag="ps_S")
            s_hi = ps_S.tile([128, 2, Q + 1], F32, tag="ps_S")
            s_views = [s_lo[:, 0, :], s_lo[:, 1, :], s_hi[:, 0, :], s_hi[:, 1, :]]
            for t in range(CT):
                for k in range(KT):
                    nc.tensor.matmul(
                        s_views[t],
                        lhsT=ctxT_sb[:, k, t * 128 : (t + 1) * 128],
                        rhs=rhs_ext[:, k, :],
                        start=(k == 0),
                        stop=False,
                    )
                nc.tensor.matmul(
                    s_views[t], lhsT=ones_row, rhs=rhs_sq, start=False, stop=True
                )

            # --- per-tile softmax stats + b-path accumulation ---
            sc4 = small_pool.tile([128, CT], F32, tag="sc4")
            m4 = small_pool.tile([128, CT], F32, tag="m4")
            em4 = small_pool.tile([128, CT], F32, tag="em4")
            zp = small_pool.tile([128, 1], F32, tag="zp")
            zs = small_pool.tile([128, 1], F32, tag="zs")
            sum4 = small_pool.tile([128, CT], F32, tag="sum4")
            rs4 = small_pool.tile([128, CT], F32, tag="rs4", name=f"rs4_{b}")
            st["rs4"] = rs4
            expS = small_pool.tile([128, CT, Q], F32, tag="expS", name=f"expS{b}")
            st["expS"] = expS
            ac_ps = ps_b.tile([128, H], F32, tag="ps_b")

            for t in range(CT):
                nc.vector.tensor_copy(sc4[:, t : t + 1], s_views[t][:, Q : Q + 1])
                nc.vector.reduce_max(
                    out=m4[:, t : t + 1],
                    in_=s_views[t][:, 0:Q],
                    axis=mybir.AxisListType.X,
                )
                nc.scalar.activation(
                    out=expS[:, t, :],
                    in_=s_views[t][:, 0:Q],
                    func=AF.Exp,
                    bias=sc4[:, t : t + 1],
                    accum_out=sum4[:, t : t + 1],
                )
                # em_t = exp(max_q S_t + s_c_t); ac accumulates immediately
                nc.scalar.activation(
                    out=em4[:, t : t + 1],
                    in_=m4[:, t : t + 1],
                    func=AF.Exp,
                    bias=sc4[:, t : t + 1],
                )
                em_b = em4[:, t : t + 1].to_broadcast([128, 128])
                nc.tensor.matmul(
                    ac_ps,
                    lhsT=em_b,
                    rhs=ctx_sb[:, t, :],
                    start=(t == 0),
                    stop=(t == CT - 1),
                )
            nc.vector.reciprocal(rs4, sum4)

            # Z = sum_c em; 1/Z via GpSimd partition all-reduce
            nc.vector.reduce_sum(out=zp, in_=em4, axis=mybir.AxisListType.X)
            nc.gpsimd.partition_all_reduce(
                zs, zp, channels=128, reduce_op=bass_isa.ReduceOp.add
            )
            rz128 = small_pool.tile([128, 1], F32, tag="rz128")
            nc.vector.reciprocal(rz128, zs)
            bc_sb = small_pool.tile([128, H], F32, tag="bc_sb", name=f"bc_sb{b}")
            st["bc_sb"] = bc_sb
            nc.vector.tensor_scalar_mul(out=bc_sb, in0=ac_ps, scalar1=rz128)
            return st

        def stage_late(b, st):
            g_v, ctx_sb = st["g_v"], st["ctx_sb"]
            expS, rs4, bc_sb, q_sb = st["expS"], st["rs4"], st["bc_sb"], st["q_sb"]

            # --- a^T (unnormalized) ---
            at_ps = ps_small.tile([Q, C], F32, tag="ps_misc")
            for t in range(CT):
                nc.tensor.transpose(
                    at_ps[:, t * 128 : (t + 1) * 128], expS[:, t, :], ident
                )
            aT_sb = small_pool.tile([Q, C], F32, tag="aT_sb")
            nc.scalar.copy(out=aT_sb, in_=at_ps)

            # --- G4 first: only needs bc_sb, flows while aq matmuls run ---
            g234s = []
            for t in range(CT):
                g234 = g_pool.tile([128, 3 * H], F32, tag="g234", name=f"g234_{b}_{t}")
                g234s.append(g234)
                nc.vector.tensor_mul(
                    out=g234[:, 2 * H : 3 * H], in0=ctx_sb[:, t, :], in1=bc_sb
                )
                nc.gpsimd.dma_start(
                    out=g_v[:, t, 3 * H : 4 * H], in_=g234[:, 2 * H : 3 * H]
                )

            # --- attended_query + G2/G3, one c-tile at a time ---
            for t in range(CT):
                aq_ps = ps_aq.tile([128, H], F32, tag="ps_aq")
                nc.tensor.matmul(
                    aq_ps,
                    lhsT=aT_sb[:, t * 128 : (t + 1) * 128],
                    rhs=q_sb[:, :],
                    start=True,
                    stop=True,
                )
                g234 = g234s[t]
                # G2 = aq / rowsum  (normalization folded into evacuation)
                nc.scalar.activation(
                    out=g234[:, 0:H], in_=aq_ps, func=AF.Copy, scale=rs4[:, t : t + 1]
                )
                # G3 = ctx * aq
                nc.vector.tensor_mul(
                    out=g234[:, H : 2 * H], in0=ctx_sb[:, t, :], in1=g234[:, 0:H]
                )
                nc.sync.dma_start(out=g_v[:, t, H : 3 * H], in_=g234[:, 0 : 2 * H])

        sts = {}
        for b in range(BPC + 1):
            if b < BPC:
                sts[b] = stage_early(b)
            if b >= 1:
                stage_late(b - 1, sts.pop(b - 1))

    nc.compile()
    return nc


_NC_CACHE = None


def kernel(context: np.ndarray, query: np.ndarray, W: np.ndarray) -> np.ndarray:
    global _NC_CACHE
    if _NC_CACHE is None:
        _NC_CACHE = build_nc()
    nc = _NC_CACHE

    context = np.ascontiguousarray(context, dtype=np.float32)
    query = np.ascontiguousarray(query, dtype=np.float32)
    W = np.ascontiguousarray(W, dtype=np.float32)

    in_maps = [
        {
            "context": context[i * BPC : (i + 1) * BPC],
            "query": query[i * BPC : (i + 1) * BPC],
            "W": W,
        }
        for i in range(NCORES)
    ]
    res = run_bass_kernel_spmd(nc, in_maps, core_ids=list(range(NCORES)))
    return np.concatenate([r["G"] for r in res.results], axis=0)



# revision 10
# speedup vs baseline: 1.2023x; 1.2023x over previous
"""BiDAF attention kernel for Trainium2, data-parallel over batch on 8 NeuronCores.

Reference math (per batch b):
    S = (ctx * w_m) @ query^T + ctx@w_c [:,None] + query@w_q [None,:]   [C, Q]
    a = softmax(S, axis=q);       attended_query    aq = a @ query       [C, H]
    m = max(S, axis=q); bweights = softmax(m, axis=c)
    attended_context ac = bweights @ ctx                                  [H]
    G = concat([ctx, aq, ctx*aq, ctx*ac[None,:]], axis=-1)               [C, 4H]

The kernel is HBM-DMA-bound (~16.5MiB out + 4.5MiB in per core), so the
design minimizes DMA-engine occupancy and keeps the transfer queue fed:
  - All on-chip compute in bf16.  Inputs are cast fp32->bf16 *during* the
    DMA load, which halves the load's DMA-engine occupancy; G2/3/4 are
    produced as one bf16 [128, 3H] tile per c-tile and stored with a single
    casting DMA (bf16->fp32).
  - G[:, 0:H] == ctx is emitted as a DRAM->DRAM copy: exact fp32, no SBUF
    round-trip, and dependency-free so it fills the DMA pipe while the
    first batches compute.
  - Every DMA goes through the gpsimd SWDGE stream so arrival order at the
    DMA engines is exactly emission order: critical loads first, G1 filler
    next, then per-batch stores as compute produces them.
  - W is loaded as [12, 128] rows (contiguous 512B descriptors) and
    transposed on PE, instead of a 4-byte-strided gather.
  - Softmax skips max-subtraction (|S| <= ~8 for this input distribution);
    exp runs on ScalarE with bias=s_c and accum_out giving the row sums.
    a is renormalized lazily at PSUM evacuation; the b-path matmul uses a
    stride-0 broadcast of exp(rowmax+s_c) so attended_context lands already
    broadcast to [128, H].
"""

import numpy as np
from contextlib import ExitStack

import concourse.bass as bass
import concourse.bacc as bacc
import concourse.bass_isa as bass_isa
import concourse.tile as tile
from concourse import mybir
from concourse.bass_utils import run_bass_kernel_spmd
from concourse.masks import make_identity

F32 = mybir.dt.float32
BF16 = mybir.dt.bfloat16
AF = mybir.ActivationFunctionType

B, C, Q, H = 32, 512, 64, 512
NCORES = 8
BPC = B // NCORES  # batches per core
CT = C // 128  # c tiles
KT = H // 128  # contraction chunks


def build_nc():
    nc = bacc.Bacc("TRN2", target_bir_lowering=False, debug=False)
    ctx_d = nc.dram_tensor("context", [BPC, C, H], F32, kind="ExternalInput")
    qry_d = nc.dram_tensor("query", [BPC, Q, H], F32, kind="ExternalInput")
    w_d = nc.dram_tensor("W", [3 * H], F32, kind="ExternalInput")
    g_d = nc.dram_tensor("G", [BPC, C, 4 * H], F32, kind="ExternalOutput")

    with tile.TileContext(nc) as tc, ExitStack() as ex:
        consts = ex.enter_context(tc.tile_pool(name="consts", bufs=1))
        ctx_pool = ex.enter_context(tc.tile_pool(name="ctx", bufs=4))
        ctxT_pool = ex.enter_context(tc.tile_pool(name="ctxT", bufs=2))
        q_pool = ex.enter_context(tc.tile_pool(name="q", bufs=4))
        small_pool = ex.enter_context(tc.tile_pool(name="small", bufs=3))
        g_pool = ex.enter_context(tc.tile_pool(name="g", bufs=8))
        ps_ctxT = ex.enter_context(tc.tile_pool(name="ps_ctxT", bufs=2, space="PSUM"))
        ps_S = ex.enter_context(tc.tile_pool(name="ps_S", bufs=2, space="PSUM"))
        ps_aq = ex.enter_context(tc.tile_pool(name="ps_aq", bufs=1, space="PSUM"))
        ps_small = ex.enter_context(tc.tile_pool(name="ps_small", bufs=2, space="PSUM"))
        ps_b = ex.enter_context(tc.tile_pool(name="ps_b", bufs=1, space="PSUM"))

        # --- load phase: casting loads in a hand-ordered SWDGE stream so the
        # DMA engines never idle and batch-0 compute starts ASAP ---
        ctx_all = ctx_pool.tile([128, BPC, CT, H], BF16, tag="ctx16")
        ctx_in = ctx_d.rearrange("b (t p) d -> p b t d", p=128)
        nc.gpsimd.dma_start(out=ctx_all[:, 0], in_=ctx_in[:, 0])
        q_all = q_pool.tile([Q, BPC, H], BF16, tag="q16")
        nc.gpsimd.dma_start(out=q_all, in_=qry_d.rearrange("b q d -> q b d"))
        nc.gpsimd.dma_start(out=ctx_all[:, 1], in_=ctx_in[:, 1])

        # identity is Pool work too: slot it between load descriptor-gens
        # (PE first needs it ~2us after the query tile lands)
        ident = consts.tile([128, 128], BF16)
        make_identity(nc, ident)
        ones_row = consts.tile([1, 128], BF16)
        nc.vector.memset(ones_row, 1.0)

        nc.gpsimd.dma_start(out=ctx_all[:, 2], in_=ctx_in[:, 2])
        # G1 = ctx, exact fp32, DRAM->DRAM: keeps the DMA engines fed while
        # the first batches compute.  Two halves so the first half's
        # descriptor-gen completes before the load transfers drain.
        nc.gpsimd.dma_start(out=g_d[0:2, :, 0:H], in_=ctx_d[0:2, :, :])
        nc.gpsimd.dma_start(out=ctx_all[:, 3], in_=ctx_in[:, 3])
        nc.gpsimd.dma_start(out=g_d[2:4, :, 0:H], in_=ctx_d[2:4, :, :])

        # W rides the HWDGE/SP path (17ns transfer; keeps its descriptor-gen
        # off the Pool stream), then a tiny DVE cast to bf16
        w_rows32 = consts.tile([12, 128], F32)
        nc.sync.dma_start(out=w_rows32, in_=w_d[:].rearrange("(g p) -> g p", p=128))
        w_rows = consts.tile([12, 128], BF16)
        nc.vector.tensor_copy(w_rows, w_rows32)

        ctx16s = [ctx_all[:, b] for b in range(BPC)]
        q16s = [q_all[:, b] for b in range(BPC)]
        g_vs = [g_d[b].rearrange("(t p) f -> p t f", p=128) for b in range(BPC)]

        # wsb16[p, g]: cols 0:4 w_c, 4:8 w_q, 8:12 w_m chunks
        wt_ps = ps_small.tile([128, 12], BF16, tag="ps_misc")
        nc.tensor.transpose(wt_ps, w_rows, ident[:12, :12])
        wsb16 = consts.tile([128, 12], BF16)
        nc.vector.tensor_copy(wsb16, wt_ps)
        # fp32 copy for tensor_scalar ops (scalar1 must be fp32)
        wsb32 = consts.tile([128, 12], F32)
        nc.vector.tensor_copy(wsb32, wt_ps)

        def stage_early(b):
            st = {}
            ctx16, q16 = ctx16s[b], q16s[b]
            st["ctx16"], st["q16"], st["g_v"] = ctx16, q16, g_vs[b]

            # --- query transpose + scaled rhs build ---
            qt_ps = ps_small.tile([128, KT * Q], BF16, tag="ps_misc")
            for k in range(KT):
                nc.tensor.transpose(
                    qt_ps[:, k * Q : (k + 1) * Q],
                    q16[:, k * 128 : (k + 1) * 128],
                    ident[:Q, :Q],
                )
            qT16 = small_pool.tile([128, KT * Q], BF16, tag="qT16")
            nc.vector.tensor_copy(qT16, qt_ps)

            # rhs_ext[:, k, 0:64] = qT_k * w_m_k ; [:, k, 64] = w_c_k
            rhs_ext = small_pool.tile([128, KT, Q + 1], BF16, tag="rhs_ext")
            for k in range(KT):
                nc.vector.tensor_scalar_mul(
                    out=rhs_ext[:, k, 0:Q],
                    in0=qT16[:, k * Q : (k + 1) * Q],
                    scalar1=wsb32[:, 8 + k : 9 + k],
                )
                nc.vector.tensor_copy(rhs_ext[:, k, Q : Q + 1], wsb16[:, k : k + 1])

            # s_q^T = w_q . qT  -> [1, Q]
            sq_ps = ps_small.tile([1, Q], F32, tag="ps_misc")
            for k in range(KT):
                nc.tensor.matmul(
                    sq_ps,
                    lhsT=wsb16[:, 4 + k : 5 + k],
                    rhs=qT16[:, k * Q : (k + 1) * Q],
                    start=(k == 0),
                    stop=(k == KT - 1),
                )
            rhs_sq = small_pool.tile([1, Q + 1], BF16, tag="rhs_sq")
            nc.vector.memset(rhs_sq, 0.0)
            nc.vector.tensor_copy(rhs_sq[:, 0:Q], sq_ps)

            # --- context transpose: ctxT16[:, k, :] = ctx[:, :, k-chunk]^T ---
            ctxT16 = ctxT_pool.tile([128, KT, C], BF16, tag="ctxT16")
            for k in range(KT):
                tps = ps_ctxT.tile([128, C], BF16, tag="ps_ctxT")
                for t in range(CT):
                    nc.tensor.transpose(
                        tps[:, t * 128 : (t + 1) * 128],
                        ctx16[:, t, k * 128 : (k + 1) * 128],
                        ident,
                    )
                if k % 2 == 0:
                    nc.scalar.copy(out=ctxT16[:, k, :], in_=tps)
                else:
                    nc.vector.tensor_copy(ctxT16[:, k, :], tps)

            # --- S matmuls: S[c, 0:64] = s_m + s_q ; S[c, 64] = s_c ---
            s_lo = ps_S.tile([128, 2, Q + 1], F32, tag="ps_S")
            s_hi = ps_S.tile([128, 2, Q + 1], F32, tag="ps_S")
            s_views = [s_lo[:, 0, :], s_lo[:, 1, :], s_hi[:, 0, :], s_hi[:, 1, :]]
            for t in range(CT):
                for k in range(KT):
                    nc.tensor.matmul(
                        s_views[t],
                        lhsT=ctxT16[:, k, t * 128 : (t + 1) * 128],
                        rhs=rhs_ext[:, k, :],
                        start=(k == 0),
                        stop=False,
                    )
                nc.tensor.matmul(
                    s_views[t], lhsT=ones_row, rhs=rhs_sq, start=False, stop=True
                )

            # --- per-tile softmax stats + b-path accumulation ---
            sc4 = small_pool.tile([128, CT], F32, tag="sc4")
            m4 = small_pool.tile([128, CT], F32, tag="m4")
            em4 = small_pool.tile([128, CT], BF16, tag="em4")
            zp = small_pool.tile([128, 1], F32, tag="zp")
            zs = small_pool.tile([128, 1], F32, tag="zs")
            sum4 = small_pool.tile([128, CT], F32, tag="sum4")
            rs4 = small_pool.tile([128, CT], F32, tag="rs4", name=f"rs4_{b}")
            st["rs4"] = rs4
            expS = small_pool.tile([128, CT, Q], BF16, tag="expS", name=f"expS{b}")
            st["expS"] = expS
            ac_ps = ps_b.tile([128, H], F32, tag="ps_b")

            for t in range(CT):
                nc.vector.tensor_copy(sc4[:, t : t + 1], s_views[t][:, Q : Q + 1])
                nc.vector.reduce_max(
                    out=m4[:, t : t + 1],
                    in_=s_views[t][:, 0:Q],
                    axis=mybir.AxisListType.X,
                )
                nc.scalar.activation(
                    out=expS[:, t, :],
                    in_=s_views[t][:, 0:Q],
                    func=AF.Exp,
                    bias=sc4[:, t : t + 1],
                    accum_out=sum4[:, t : t + 1],
                )
                # em_t = exp(max_q S_t + s_c_t); ac accumulates immediately
                nc.scalar.activation(
                    out=em4[:, t : t + 1],
                    in_=m4[:, t : t + 1],
                    func=AF.Exp,
                    bias=sc4[:, t : t + 1],
                )
                em_b = em4[:, t : t + 1].to_broadcast([128, 128])
                nc.tensor.matmul(
                    ac_ps,
                    lhsT=em_b,
                    rhs=ctx16[:, t, :],
                    start=(t == 0),
                    stop=(t == CT - 1),
                )
            nc.vector.reciprocal(rs4, sum4)

            # Z = sum_c em; 1/Z via GpSimd partition all-reduce
            nc.vector.reduce_sum(out=zp, in_=em4, axis=mybir.AxisListType.X)
            nc.gpsimd.partition_all_reduce(
                zs, zp, channels=128, reduce_op=bass_isa.ReduceOp.add
            )
            rz128 = small_pool.tile([128, 1], F32, tag="rz128")
            nc.vector.reciprocal(rz128, zs)
            bc16 = small_pool.tile([128, H], BF16, tag="bc16", name=f"bc16_{b}")
            st["bc16"] = bc16
            nc.vector.tensor_scalar_mul(out=bc16, in0=ac_ps, scalar1=rz128)
            return st

        def stage_late(b, st):
            g_v, ctx16 = st["g_v"], st["ctx16"]
            expS, rs4, bc16, q16 = st["expS"], st["rs4"], st["bc16"], st["q16"]

            # --- a^T (unnormalized) ---
            at_ps = ps_small.tile([Q, C], BF16, tag="ps_misc")
            for t in range(CT):
                nc.tensor.transpose(
                    at_ps[:, t * 128 : (t + 1) * 128], expS[:, t, :], ident
                )
            aT16 = small_pool.tile([Q, C], BF16, tag="aT16")
            nc.scalar.copy(out=aT16, in_=at_ps)

            # --- attended_query + fused G2/G3/G4 tile, one c-tile at a time ---
            for t in range(CT):
                aq_ps = ps_aq.tile([128, H], F32, tag="ps_aq")
                nc.tensor.matmul(
                    aq_ps,
                    lhsT=aT16[:, t * 128 : (t + 1) * 128],
                    rhs=q16[:, :],
                    start=True,
                    stop=True,
                )
                g234 = g_pool.tile([128, 3 * H], BF16, tag="g234", name=f"g234_{b}_{t}")
                # G2 = aq / rowsum  (normalization folded into evacuation)
                nc.scalar.activation(
                    out=g234[:, 0:H], in_=aq_ps, func=AF.Copy, scale=rs4[:, t : t + 1]
                )
                # G3 = ctx * aq
                nc.vector.tensor_mul(
                    out=g234[:, H : 2 * H], in0=ctx16[:, t, :], in1=g234[:, 0:H]
                )
                # G4 = ctx * attended_context
                nc.vector.tensor_mul(
                    out=g234[:, 2 * H : 3 * H], in0=ctx16[:, t, :], in1=bc16
                )
                # single casting store bf16 -> fp32 covers G[:, H:4H]
                nc.gpsimd.dma_start(out=g_v[:, t, H : 4 * H], in_=g234)

        for b in range(BPC):
            stage_late(b, stage_early(b))

    nc.compile()
    return nc


_NC_CACHE = None


def kernel(context: np.ndarray, query: np.ndarray, W: np.ndarray) -> np.ndarray:
    global _NC_CACHE
    if _NC_CACHE is None:
        _NC_CACHE = build_nc()
    nc = _NC_CACHE

    context = np.ascontiguousarray(context, dtype=np.float32)
    query = np.ascontiguousarray(query, dtype=np.float32)
    W = np.ascontiguousarray(W, dtype=np.float32)

    in_maps = [
        {
            "context": context[i * BPC : (i + 1) * BPC],
            "query": query[i * BPC : (i + 1) * BPC],
            "W": W,
        }
        for i in range(NCORES)
    ]
    res = run_bass_kernel_spmd(nc, in_maps, core_ids=list(range(NCORES)))
    return np.concatenate([r["G"] for r in res.results], axis=0)
